# revision 46
# baseline (speedup 1.0000x reference)
"""MoE expert-parallel SwiGLU MLP kernel for 8 TRN2 NeuronCores.

Problem (nn_Experts): E=8 experts, each computes, for its [G=2048, D=1024]
token slice x and weights w_in/w_swiglu [D, F=4096], w_out [F, D]:

    hidden = silu(x @ w_in) * (x @ w_swiglu)
    out    = hidden @ w_out

Sharding: expert-parallel, one expert per NeuronCore (SPMD — same program,
per-core input slices). No cross-device comms.

Per-core kernel design (PE-roofline ~655us of 3072 512-col matmuls):
  - All matmuls in bf16 (full PE rate; fp32 is 1/4 rate) with fp32 PSUM accum.
  - x is cast fp32->bf16 on DVE/Act, then transposed on the PE (128x128
    identity-transpose, 1 cycle/row in bf16) into xT[d, g]; 4 transposes
    share one PSUM tile and copy back to SBUF in a single batched copy.
  - Phase A (per 1024-token g-block): for each f-tile, mid/gate psum tiles are
    produced by 8-step d-accumulation; Silu on ScalarE, multiply on DVE writes
    hiddenT[f, g] in bf16.
  - Phase B: out[g, d] accumulates 32 f-steps with hiddenT 128x128 tiles as
    the stationary operand and resident bf16 w_out as the moving operand;
    PSUM -> SBUF copy, then DMA to DRAM.

  - Transpose casts/copybacks pinned to DVE (tr_dve); w_out residency
    loads shifted 4 f-iterations later (wob_delay) to clear startup DMA.

Measured (repeat-body slope, interleaved R=1 vs R=17 pairs): ~703us/core,
down from 785us baseline; paired A/B vs session-start baseline: -34us
+/- 10. Cost-model prediction 679us; bf16 PE floor 655us.
Dead ends (measured): ldw_dedup slower (HW overlaps LDWEIGHTS already),
pair_mm neutral, XBAR dma-transpose (full or gb1-only) starves PE or
weight DMAs, DMA issue on the Activation HWDGE queue stalls silu,
gs_defer neutral (startup is x-DMA-feed-bound), fp8 DoubleRow fails the
2e-2 accuracy gate.
"""

import numpy as np

import concourse.bass as bass  # noqa: F401  (AP helpers)
import concourse.mybir as mybir
import concourse.tile as tile
from concourse import bacc
from concourse.bass_utils import run_bass_kernel_spmd
from concourse.masks import make_identity

E = 8
G = 2048  # tokens per expert
D = 1024
F = 4096
P = 128
NB = 512  # matmul moving free dim (one PSUM bank of fp32)
GB = 1024  # g-block
N_GB = G // GB  # 2
DT = D // P  # 8 d-tiles
FT = F // P  # 32 f-tiles

F32 = mybir.dt.float32
BF16 = mybir.dt.bfloat16

DEFAULT_CFG = dict(wst_bufs=4, wbf_bufs=3, silu_bufs=2, xst_bufs=4, wost_bufs=2,
                   pair_mm=False, wf_chunk=128, ldw_dedup=False,
                   dma_transpose=False, bf16_tr=True, psum_dma=False,
                   first_small=False, tail_split=False, tr_batch=True,
                   x_dma_act=False, tr_batch8=False, w_early=0,
                   w_dma_act=False, xbar_gb1=False, tr_dve=True,
                   wob_delay=4, gs_defer=False, x_dma_split=False,
                   silu_bf16=False,
                   skip_phaseA=False, skip_phaseB=False, skip_transpose=False,
                   tr_bufs=4, mid_bufs=2, gate_bufs=2, out_bufs=4, tr_tag="out",
                   mid_tag="mid", gate_tag="gate", out_tag="out",
                   share_xst=False)
CFG = dict(DEFAULT_CFG)


def build_nc(repeat=1, cfg=None):
    global CFG
    CFG = dict(DEFAULT_CFG)
    if cfg:
        CFG.update(cfg)
    nc = bacc.Bacc(target_bir_lowering=False)
    x = nc.dram_tensor("x", [G, D], F32, kind="ExternalInput")
    w_in = nc.dram_tensor("w_in", [D, F], F32, kind="ExternalInput")
    w_sw = nc.dram_tensor("w_sw", [D, F], F32, kind="ExternalInput")
    w_out = nc.dram_tensor("w_out", [F, D], F32, kind="ExternalInput")
    out = nc.dram_tensor("out", [G, D], F32, kind="ExternalOutput")

    w_in_t = w_in.rearrange("(po p) f -> p po f", p=P)  # [128, 8, 4096]
    w_sw_t = w_sw.rearrange("(po p) f -> p po f", p=P)

    with tile.TileContext(nc) as tc:
        with (
            tc.tile_pool(name="const", bufs=1) as const_pool,
            tc.tile_pool(name="wob", bufs=1) as wob_pool,
            tc.tile_pool(name="xT", bufs=1) as xT_pool,
            tc.tile_pool(name="hid", bufs=1) as hid_pool,
            tc.tile_pool(name="wst", bufs=CFG["wst_bufs"]) as wst_pool,
            tc.tile_pool(name="wbf", bufs=CFG["wbf_bufs"]) as wbf_pool,
            tc.tile_pool(name="silu", bufs=CFG["silu_bufs"]) as silu_pool,
            tc.tile_pool(name="xst", bufs=CFG["xst_bufs"]) as xst_pool,
            tc.tile_pool(name="psum", bufs=2, space="PSUM") as psum_pool,
            tc.tile_pool(name="dram", bufs=1, space="DRAM") as dram_pool,
        ):
            identity = const_pool.tile([P, P], F32)
            make_identity(nc, identity)
            identity_bf = None
            if CFG["bf16_tr"]:
                identity_bf = const_pool.tile([P, P], BF16, name="id_bf")
                make_identity(nc, identity_bf)
            for _rep in range(repeat):
                _emit_once(nc, tc, identity, x, w_in_t, w_sw_t, w_out, out,
                           wob_pool, xT_pool, hid_pool, wst_pool, wbf_pool,
                           silu_pool, xst_pool, psum_pool, dram_pool,
                           identity_bf=identity_bf)
    if CFG["ldw_dedup"]:
        nc.compile()
        n = _dedup_ldweights(nc)
        bass.Bass.finalize(nc)
        nc._ldw_removed = n
    else:
        nc.finalize()
    return nc


def _dedup_ldweights(nc):
    """Remove InstLdweights that reload the exact weights already resident in
    the PE array (same memref/offset/ap/dtype, no intervening transpose, same
    basic block). The paired InstMatmult has ldweights=False and reads the
    array state, so eliding the reload is semantics-preserving; the removed
    instruction's semaphore waits/updates move to the next PE instruction."""
    import concourse.mybir as _mybir
    PE = _mybir.EngineType.PE
    removed = 0
    fn = nc.m.functions[0]
    for bb in fn.blocks:
        insts = list(bb.instructions)
        keep = []
        last_sig = None
        pending_sync = []  # sync_infos of removed LDWs awaiting next PE inst
        for inst in insts:
            cn = inst.__class__.__name__
            is_pe = getattr(inst, "engine", None) == PE
            if cn == "InstLdweights":
                a = inst.ins[0]
                sig = (a.memref, a.offset, str(a.ap), str(a.dtype),
                       str(inst.perf_mode), str(inst.is_transpose))
                if sig == last_sig:
                    if inst.sync_info is not None and (
                        inst.sync_info.on_wait or inst.sync_info.on_update
                    ):
                        pending_sync.append(inst.sync_info)
                    removed += 1
                    continue
                last_sig = sig
            elif cn == "InstMatmult" and inst.is_transpose:
                last_sig = None  # transpose streams data through weight path
            if is_pe and pending_sync:
                si = inst.sync_info
                if si is None:
                    si = _mybir.SyncInfo(on_wait=[], on_update=[])
                    inst.sync_info = si
                for ps in pending_sync:
                    si.on_wait = list(si.on_wait) + list(ps.on_wait)
                    si.on_update = list(si.on_update) + list(ps.on_update)
                pending_sync = []
            keep.append(inst)
        if removed and len(keep) != len(insts):
            assert not pending_sync, "dangling sync from removed trailing LDW"
            bb.set_instructions(keep) if hasattr(bb, "set_instructions") else None
            if not hasattr(bb, "set_instructions"):
                # fall back: mutate in place via slice assignment if supported
                try:
                    bb.instructions = keep
                except Exception:
                    # remove one by one
                    cur = bb.instructions
                    for i in range(len(cur) - 1, -1, -1):
                        if cur[i] not in keep:
                            del cur[i]
    return removed


def _emit_once(nc, tc, identity, x, w_in_t, w_sw_t, w_out, out,
               wob_pool, xT_pool, hid_pool, wst_pool, wbf_pool,
               silu_pool, xst_pool, psum_pool, dram_pool, identity_bf=None):
    if True:
        if True:
            # Resident bf16 copy of w_out: wob[p, ft, d] = w_out[ft*128+p, d].
            # Loads are interleaved into phase A's f-loop (first g-block) so
            # the upfront DMA bandwidth goes to x / w_in / w_sw instead.
            wob = wob_pool.tile([P, FT, D], BF16, tag="wob")

            def load_wob(ft):
                wost = wst_pool.tile([P, D], F32, tag="wost", bufs=CFG["wost_bufs"],
                                     name=f"wost_{ft}")
                nc.sync.dma_start(wost[:], w_out[ft * P:(ft + 1) * P, :])
                nc.any.tensor_copy(out=wob[:, ft, :], in_=wost[:])

            # xT[p, dt, g'] = x[g, dt*128+p] in bf16 (PE transpose + cast),
            # one tile per g-block so phase A of block 0 can start after
            # only the first half of the transposes.
            xT_blocks = [
                xT_pool.tile([P, DT, GB], BF16, tag=f"xT{gb}", name=f"xT{gb}")
                for gb in range(N_GB)
            ]

            def transpose_gt(gb, gt):
                """Transpose x rows [gb*GB + gt*128, +128) into xT_blocks[gb]."""
                xTb = xT_blocks[gb]
                if CFG["bf16_tr"] and CFG["tr_batch8"]:
                    # all 8 d-tiles of one g-tile into a single PSUM bank;
                    # one batched copyback per g-tile
                    ptr8 = psum_pool.tile([P, DT, P], BF16, tag=CFG["tr_tag"],
                                          bufs=CFG["tr_bufs"], name="ptr8")
                    for dh in range(2):
                        xst = xst_pool.tile([P, NB], F32, tag="xst",
                                            name="xst")
                        nc.sync.dma_start(
                            xst[:],
                            x[gb * GB + gt * P:gb * GB + (gt + 1) * P,
                              dh * NB:(dh + 1) * NB],
                        )
                        xbf = xst_pool.tile([P, NB], BF16, tag="xbf",
                                            name="xbf")
                        nc.any.tensor_copy(out=xbf[:], in_=xst[:])
                        for dq in range(NB // P):
                            nc.tensor.transpose(
                                ptr8[:, dh * (NB // P) + dq, :],
                                xbf[:, dq * P:(dq + 1) * P],
                                identity_bf[:]
                            )
                    nc.any.tensor_copy(
                        out=xTb[:, :, gt * P:(gt + 1) * P], in_=ptr8[:]
                    )
                    return
                for dh in range(2):  # two 512-col halves of the d axis
                    first_small = (CFG["bf16_tr"] and CFG["first_small"]
                                   and gb == 0 and gt == 0)
                    if not first_small:
                        xst = xst_pool.tile([P, NB], F32, tag="xst",
                                            name="xst")
                        if CFG["x_dma_split"]:
                            # alternate gb0 x DMAs across both HWDGE queues
                            # (SP + Act) to double startup issue rate; gb1
                            # transposes run during phase A where Act is busy
                            dma_eng = (nc.scalar if (gb == 0 and gt % 2)
                                       else nc.sync)
                        elif CFG["x_dma_act"]:
                            dma_eng = nc.scalar
                        else:
                            dma_eng = nc.sync
                        dma_eng.dma_start(
                            xst[:],
                            x[gb * GB + gt * P:gb * GB + (gt + 1) * P,
                              dh * NB:(dh + 1) * NB],
                        )
                    if CFG["bf16_tr"]:
                        if first_small:
                            # chunked first tile: shortest possible chain to
                            # the first PE transpose (cuts kernel lead-in)
                            for dq in range(NB // P):
                                dt = dh * (NB // P) + dq
                                xstc = xst_pool.tile([P, P], F32, tag="xst",
                                                     name="xst_c")
                                nc.sync.dma_start(
                                    xstc[:],
                                    x[gb * GB + gt * P:gb * GB + (gt + 1) * P,
                                      dt * P:(dt + 1) * P],
                                )
                                xbfc = xst_pool.tile([P, P], BF16, tag="xbf",
                                                     name="xbf_c")
                                nc.any.tensor_copy(out=xbfc[:], in_=xstc[:])
                                ptr = psum_pool.tile([P, P], BF16,
                                                     tag=CFG["tr_tag"],
                                                     bufs=CFG["tr_bufs"],
                                                     name="ptr")
                                nc.tensor.transpose(ptr[:], xbfc[:],
                                                    identity_bf[:])
                                nc.any.tensor_copy(
                                    out=xTb[:, dt, gt * P:(gt + 1) * P],
                                    in_=ptr[:]
                                )
                            continue
                        # cast once on DVE/Act, then 1-cycle/row bf16 PE
                        # transposes (fp32 transpose is 2 cycles/row)
                        cp_eng = nc.vector if CFG["tr_dve"] else nc.any
                        xbf = xst_pool.tile([P, NB], BF16, tag="xbf",
                                            name="xbf")
                        cp_eng.tensor_copy(out=xbf[:], in_=xst[:])
                        if CFG["tr_batch"]:
                            # 4 transposes share one PSUM tile; single
                            # batched copyback (fewer insts + sems)
                            ptr4 = psum_pool.tile([P, NB // P, P], BF16,
                                                  tag=CFG["tr_tag"],
                                                  bufs=CFG["tr_bufs"],
                                                  name="ptr4")
                            for dq in range(NB // P):
                                nc.tensor.transpose(
                                    ptr4[:, dq, :],
                                    xbf[:, dq * P:(dq + 1) * P],
                                    identity_bf[:]
                                )
                            cp_eng.tensor_copy(
                                out=xTb[:, dh * (NB // P):(dh + 1) * (NB // P),
                                        gt * P:(gt + 1) * P],
                                in_=ptr4[:]
                            )
                            continue
                        for dq in range(NB // P):
                            dt = dh * (NB // P) + dq
                            ptr = psum_pool.tile([P, P], BF16,
                                                 tag=CFG["tr_tag"],
                                                 bufs=CFG["tr_bufs"],
                                                 name="ptr")
                            nc.tensor.transpose(
                                ptr[:], xbf[:, dq * P:(dq + 1) * P],
                                identity_bf[:]
                            )
                            nc.any.tensor_copy(
                                out=xTb[:, dt, gt * P:(gt + 1) * P], in_=ptr[:]
                            )
                        continue
                    for dq in range(NB // P):
                        dt = dh * (NB // P) + dq
                        ptr = psum_pool.tile([P, P], F32, tag=CFG["tr_tag"],
                                             bufs=CFG["tr_bufs"], name="ptr")
                        nc.tensor.transpose(
                            ptr[:], xst[:, dq * P:(dq + 1) * P], identity[:]
                        )
                        nc.any.tensor_copy(
                            out=xTb[:, dt, gt * P:(gt + 1) * P], in_=ptr[:]
                        )

            xbar_dram = {}

            def stage_xbar_gt(gb, gt):
                """x rows of one g-tile -> bf16 DRAM scratch (XBAR source)."""
                if gb not in xbar_dram:
                    xbar_dram[gb] = dram_pool.tile([GB, D], BF16,
                                                   tag=f"xbars{gb}",
                                                   name=f"xbars{gb}")
                xd = xbar_dram[gb]
                for dh in range(2):
                    xst = xst_pool.tile([P, NB], F32, tag="xst", name="xst")
                    nc.sync.dma_start(
                        xst[:],
                        x[gb * GB + gt * P:gb * GB + (gt + 1) * P,
                          dh * NB:(dh + 1) * NB],
                    )
                    xbf = xst_pool.tile([P, NB], BF16, tag="xbf", name="xbf")
                    nc.any.tensor_copy(out=xbf[:], in_=xst[:])
                    nc.sync.dma_start(
                        xd[gt * P:(gt + 1) * P, dh * NB:(dh + 1) * NB],
                        xbf[:]
                    )

            def xbar_read_dt(gb, dt):
                """XBAR-transposed read of one d-tile into xT_blocks[gb]."""
                nc.sync.dma_start_transpose(
                    xT_blocks[gb][:, dt, :],
                    xbar_dram[gb][:, dt * P:(dt + 1) * P],
                )

            def dma_transpose_block(gb):
                """x rows of block gb -> bf16 DRAM scratch -> XBAR-transposed
                reads into xT_blocks[gb]. No PE involvement."""
                xbf_dram = dram_pool.tile([GB, D], BF16, tag=f"xbf{gb}",
                                          name=f"xbf{gb}")
                for gt in range(GB // P):
                    xst = xst_pool.tile([P, D], F32, tag="xst", name="xst")
                    nc.sync.dma_start(
                        xst[:], x[gb * GB + gt * P:gb * GB + (gt + 1) * P, :]
                    )
                    xbf_sb = xst_pool.tile([P, D], BF16, tag="xbf_sb",
                                           name="xbf_sb")
                    nc.any.tensor_copy(out=xbf_sb[:], in_=xst[:])
                    nc.sync.dma_start(xbf_dram[gt * P:(gt + 1) * P, :], xbf_sb[:])
                for dt in range(DT):
                    nc.sync.dma_start_transpose(
                        xT_blocks[gb][:, dt, :],
                        xbf_dram[:, dt * P:(dt + 1) * P],
                    )

            WFC0 = CFG["wf_chunk"]

            def load_chunk(f0):
                w_eng = nc.scalar if CFG["w_dma_act"] else nc.sync
                wst_i = wst_pool.tile([P, DT, WFC0], F32, tag="wst")
                w_eng.dma_start(wst_i[:], w_in_t[:, :, f0:f0 + WFC0])
                wbf_ci = wbf_pool.tile([P, DT, WFC0], BF16, tag="wbf")
                nc.any.tensor_copy(out=wbf_ci[:], in_=wst_i[:])
                wst_s = wst_pool.tile([P, DT, WFC0], F32, tag="wst")
                w_eng.dma_start(wst_s[:], w_sw_t[:, :, f0:f0 + WFC0])
                wbf_cs = wbf_pool.tile([P, DT, WFC0], BF16, tag="wbf")
                nc.any.tensor_copy(out=wbf_cs[:], in_=wst_s[:])
                return {"i": wbf_ci, "s": wbf_cs}

            prefetched = {}

            if CFG["skip_transpose"]:
                for gb in range(N_GB):
                    nc.any.memzero(xT_blocks[gb][:])
            elif CFG["dma_transpose"] == "gb1":
                for gt in range(GB // P):
                    transpose_gt(0, gt)
                dma_transpose_block(1)
            elif CFG["dma_transpose"]:
                for gb in range(N_GB):
                    dma_transpose_block(gb)
            else:
                # g-block 0 upfront; g-block 1 interleaved into phase A below.
                # With gs_defer, only gt0-3 upfront — gt4-7 are emitted inside
                # phase A after the first gs=0 matmul groups.
                defer_on = (CFG["gs_defer"] and not CFG["pair_mm"]
                            and not CFG["skip_phaseA"])
                n_up = 4 if defer_on else GB // P
                for gt in range(n_up):
                    if gt == 2:
                        for c in range(CFG["w_early"]):
                            prefetched[c] = load_chunk(c * WFC0)
                    transpose_gt(0, gt)

            for gb in range(N_GB):
                # hidT[p, ft, g'] = hidden[gb*GB+g', ft*128+p] in bf16
                hidT = hid_pool.tile([P, FT, GB], BF16, tag="hid")

                # Phase A: mid/gate matmuls + SwiGLU -> hidT
                if CFG["skip_phaseA"]:
                    nc.any.memzero(hidT[:])
                    if gb == 0:
                        for ft in range(FT):
                            load_wob(ft)
                WFC = CFG["wf_chunk"]  # f-width of one w_in/w_sw DMA chunk
                FPC = WFC // P  # f-tiles per chunk
                chunks = {}

                def get_wbf(ft):
                    ci = ft // FPC
                    if ci not in chunks:
                        if gb == 0 and ci in prefetched:
                            chunks[ci] = prefetched.pop(ci)
                        else:
                            chunks[ci] = load_chunk(ci * WFC)
                    fo = (ft % FPC) * P
                    return (chunks[ci]["i"][:, :, fo:fo + P],
                            chunks[ci]["s"][:, :, fo:fo + P])

                def ft_side_effects(ft):
                    if gb != 0:
                        return
                    if ft >= CFG["wob_delay"]:
                        load_wob(ft - CFG["wob_delay"])
                    if (not CFG["skip_transpose"]
                            and not CFG["dma_transpose"]):
                        if CFG["xbar_gb1"]:
                            # gb1 x -> bf16 DRAM scratch over ft 0..15,
                            # XBAR-transposed reads over ft 16..31: no
                            # PE involvement for g-block 1's transpose
                            if ft % 2 == 0 and ft < 16:
                                stage_xbar_gt(1, ft // 2)
                            elif ft % 2 == 0:
                                xbar_read_dt(1, (ft - 16) // 2)
                        elif ft % 4 == 0:
                            # 8 remaining transposes spread over the f-loop
                            transpose_gt(1, ft // 4)

                def a_step(ft, gs):
                    wbf_i, wbf_s = get_wbf(ft)
                    xT = xT_blocks[gb]
                    g0 = gs * NB
                    mid_ps = psum_pool.tile([P, NB], F32, tag=CFG["mid_tag"],
                                            bufs=CFG["mid_bufs"])
                    for dt in range(DT):
                        nc.tensor.matmul(
                            mid_ps[:],
                            wbf_i[:, dt, :],
                            xT[:, dt, g0:g0 + NB],
                            start=(dt == 0),
                            stop=(dt == DT - 1),
                        )
                    gate_ps = psum_pool.tile([P, NB], F32, tag=CFG["gate_tag"],
                                             bufs=CFG["gate_bufs"])
                    for dt in range(DT):
                        nc.tensor.matmul(
                            gate_ps[:],
                            wbf_s[:, dt, :],
                            xT[:, dt, g0:g0 + NB],
                            start=(dt == 0),
                            stop=(dt == DT - 1),
                        )
                    silu_t = silu_pool.tile(
                        [P, NB], BF16 if CFG["silu_bf16"] else F32, tag="silu")
                    nc.scalar.activation(
                        silu_t[:], mid_ps[:], mybir.ActivationFunctionType.Silu
                    )
                    nc.vector.tensor_mul(
                        out=hidT[:, ft, gs * NB:(gs + 1) * NB],
                        in0=silu_t[:],
                        in1=gate_ps[:],
                    )

                use_defer = (gb == 0 and CFG["gs_defer"]
                             and not CFG["pair_mm"]
                             and not CFG["skip_phaseA"]
                             and not CFG["skip_transpose"]
                             and not CFG["dma_transpose"])
                if use_defer:
                    # phase A starts after only gt0-3 + chunk0; gs=1 of the
                    # first two f-tiles is deferred until gt4-7 transpose
                    # while the PE chews on their gs=0 matmuls
                    sched = [(0, 0), (1, 0), (0, 1), (1, 1)]
                    sched += [(ft, gs) for ft in range(2, FT)
                              for gs in range(GB // NB)]
                    seen = set()
                    for ft, gs in sched:
                        if ft not in seen:
                            seen.add(ft)
                            get_wbf(ft)  # chunk DMA first, then side effects
                            ft_side_effects(ft)
                        a_step(ft, gs)
                        if (ft, gs) == (0, 0):
                            transpose_gt(0, 4)
                            transpose_gt(0, 5)
                        elif (ft, gs) == (1, 0):
                            transpose_gt(0, 6)
                            transpose_gt(0, 7)
                elif not CFG["skip_phaseA"] and not CFG["pair_mm"]:
                    for ft in range(FT):
                        get_wbf(ft)  # chunk DMA first, then side effects
                        ft_side_effects(ft)
                        for gs in range(GB // NB):
                            a_step(ft, gs)

                wbf_cache = {}
                for ft in (range(FT) if not CFG["skip_phaseA"]
                           and CFG["pair_mm"] else []):
                    if ft % FPC == 0:
                        ci = ft // FPC
                        if gb == 0 and ci in prefetched:
                            wbf_cache = prefetched.pop(ci)
                        else:
                            wbf_cache = load_chunk(ft * P)
                    fo = (ft % FPC) * P
                    wbf_i = wbf_cache["i"][:, :, fo:fo + P]
                    wbf_s = wbf_cache["s"][:, :, fo:fo + P]

                    ft_side_effects(ft)

                    xT = xT_blocks[gb]
                    if CFG["pair_mm"]:
                        # d-outer, gs-paired: adjacent matmuls share lhsT so
                        # walrus ldw-opt can elide every second LDWEIGHTS.
                        NGS = GB // NB
                        mids = [psum_pool.tile([P, NB], F32, tag=CFG["mid_tag"],
                                               bufs=CFG["mid_bufs"], name="mid_ps")
                                for _ in range(NGS)]
                        gates = [psum_pool.tile([P, NB], F32, tag=CFG["gate_tag"],
                                                bufs=CFG["gate_bufs"], name="gate_ps")
                                 for _ in range(NGS)]
                        for dt in range(DT):
                            for gs in range(NGS):
                                nc.tensor.matmul(
                                    mids[gs][:],
                                    wbf_i[:, dt, :],
                                    xT[:, dt, gs * NB:(gs + 1) * NB],
                                    start=(dt == 0),
                                    stop=(dt == DT - 1),
                                )
                        for dt in range(DT):
                            for gs in range(NGS):
                                nc.tensor.matmul(
                                    gates[gs][:],
                                    wbf_s[:, dt, :],
                                    xT[:, dt, gs * NB:(gs + 1) * NB],
                                    start=(dt == 0),
                                    stop=(dt == DT - 1),
                                )
                        for gs in range(NGS):
                            silu_t = silu_pool.tile([P, NB], F32, tag="silu",
                                                    name="silu_t")
                            nc.scalar.activation(
                                silu_t[:], mids[gs][:],
                                mybir.ActivationFunctionType.Silu
                            )
                            nc.vector.tensor_mul(
                                out=hidT[:, ft, gs * NB:(gs + 1) * NB],
                                in0=silu_t[:],
                                in1=gates[gs][:],
                            )
                    else:
                        for gs in range(GB // NB):  # 2 x 512 columns
                            g0 = gs * NB
                            mid_ps = psum_pool.tile([P, NB], F32, tag=CFG["mid_tag"], bufs=CFG["mid_bufs"])
                            for dt in range(DT):
                                nc.tensor.matmul(
                                    mid_ps[:],
                                    wbf_i[:, dt, :],
                                    xT[:, dt, g0:g0 + NB],
                                    start=(dt == 0),
                                    stop=(dt == DT - 1),
                                )
                            gate_ps = psum_pool.tile([P, NB], F32, tag=CFG["gate_tag"], bufs=CFG["gate_bufs"])
                            for dt in range(DT):
                                nc.tensor.matmul(
                                    gate_ps[:],
                                    wbf_s[:, dt, :],
                                    xT[:, dt, g0:g0 + NB],
                                    start=(dt == 0),
                                    stop=(dt == DT - 1),
                                )
                            silu_t = silu_pool.tile([P, NB], F32, tag="silu")
                            nc.scalar.activation(
                                silu_t[:], mid_ps[:], mybir.ActivationFunctionType.Silu
                            )
                            nc.vector.tensor_mul(
                                out=hidT[:, ft, gs * NB:(gs + 1) * NB],
                                in0=silu_t[:],
                                in1=gate_ps[:],
                            )

                if gb == 0 and not CFG["skip_phaseA"] and CFG["wob_delay"]:
                    # backlog of delayed wob loads; phase B bank 0 consumes
                    # wob[ft] only after ft*213ns of accumulation
                    for j in range(FT - CFG["wob_delay"], FT):
                        load_wob(j)

                # Phase B: out[g, d] = hiddenT.T @ w_out
                if CFG["skip_phaseB"]:
                    # still consume hidT minimally so it isn't dead
                    out_sb = silu_pool.tile([P, NB], F32, tag="silu", name="dummy_out")
                    nc.any.tensor_copy(out=out_sb[:], in_=hidT[:, 0, :NB])
                    nc.sync.dma_start(out[gb * GB:gb * GB + P, 0:NB], out_sb[:])
                for gt in (range(GB // P) if not CFG["skip_phaseB"] else []):  # 8 g-tiles of 128
                    g_row = gb * GB + gt * P
                    if CFG["pair_mm"]:
                        NDH = D // NB
                        outs_ps = [psum_pool.tile([P, NB], F32, tag=CFG["out_tag"],
                                                  bufs=CFG["out_bufs"], name="out_ps")
                                   for _ in range(NDH)]
                        for ft in range(FT):
                            for dh in range(NDH):
                                nc.tensor.matmul(
                                    outs_ps[dh][:],
                                    hidT[:, ft, gt * P:(gt + 1) * P],
                                    wob[:, ft, dh * NB:(dh + 1) * NB],
                                    start=(ft == 0),
                                    stop=(ft == FT - 1),
                                )
                        for dh in range(NDH):
                            out_sb = silu_pool.tile([P, NB], F32, tag="silu",
                                                    name="out_sb")
                            nc.any.tensor_copy(out=out_sb[:], in_=outs_ps[dh][:])
                            nc.sync.dma_start(
                                out[g_row:g_row + P, dh * NB:(dh + 1) * NB],
                                out_sb[:]
                            )
                    else:
                        for dh in range(D // NB):  # 2 d-halves of 512
                            out_ps = psum_pool.tile([P, NB], F32, tag=CFG["out_tag"], bufs=CFG["out_bufs"])
                            for ft in range(FT):
                                nc.tensor.matmul(
                                    out_ps[:],
                                    hidT[:, ft, gt * P:(gt + 1) * P],
                                    wob[:, ft, dh * NB:(dh + 1) * NB],
                                    start=(ft == 0),
                                    stop=(ft == FT - 1),
                                )
                            is_last = (gb == N_GB - 1 and gt == GB // P - 1
                                       and dh == D // NB - 1)
                            if CFG["tail_split"] and is_last:
                                # pipeline the final copy+DMA in 128-col
                                # chunks to shorten the kernel tail
                                for dq in range(NB // P):
                                    oc = silu_pool.tile([P, P], F32,
                                                        tag="silu",
                                                        name="out_c")
                                    nc.any.tensor_copy(
                                        out=oc[:],
                                        in_=out_ps[:, dq * P:(dq + 1) * P])
                                    nc.sync.dma_start(
                                        out[g_row:g_row + P,
                                            dh * NB + dq * P:
                                            dh * NB + (dq + 1) * P],
                                        oc[:]
                                    )
                            else:
                                out_sb = silu_pool.tile([P, NB], F32, tag="silu")
                                nc.any.tensor_copy(out=out_sb[:], in_=out_ps[:])
                                nc.sync.dma_start(
                                    out[g_row:g_row + P, dh * NB:(dh + 1) * NB], out_sb[:]
                                )


_NC_CACHE = None


def _get_nc():
    global _NC_CACHE
    if _NC_CACHE is None:
        _NC_CACHE = build_nc()
    return _NC_CACHE


def kernel(routed_in_egD, moe_w_in_eD_F, moe_w_swiglu_eD_F, moe_w_out_eF_D,
           _trace=False, _tmpdir=None):
    x = np.ascontiguousarray(np.asarray(routed_in_egD, dtype=np.float32))
    w_in = np.ascontiguousarray(np.asarray(moe_w_in_eD_F, dtype=np.float32))
    w_sw = np.ascontiguousarray(np.asarray(moe_w_swiglu_eD_F, dtype=np.float32))
    w_out = np.ascontiguousarray(np.asarray(moe_w_out_eF_D, dtype=np.float32))

    nc = _get_nc()
    in_maps = []
    for e in range(E):
        in_maps.append({
            "x": x[e * G:(e + 1) * G],
            "w_in": w_in[e * D:(e + 1) * D],
            "w_sw": w_sw[e * D:(e + 1) * D],
            "w_out": w_out[e * F:(e + 1) * F],
        })
    res = run_bass_kernel_spmd(
        nc, in_maps, core_ids=list(range(E)), trace=_trace, tmpdir=_tmpdir
    )
    out = np.concatenate([res.results[e]["out"] for e in range(E)], axis=0)
    if _trace:
        return out, res
    return out

